# revision 1
# baseline (speedup 1.0000x reference)
"""Trainium2 Bass kernel for nn_BasicRNN_42271068127787.

3-layer LSTM (input=20, hidden=6, seq=34) + FC(204->20) + log_softmax over
batch 32768, data-parallel over 8 NeuronCores (4096 rows/core).

Layout (per core):
  - batch 4096 rows -> NB=10 chunks x BF=416 cols (4160, 64 zero-pad cols)
  - feature-major on chip: activations [feature-rows, batch-cols]
  - gates per (t, layer): one PSUM tile G2 [128, 2, 416] (2 banks):
      bank0 = [i-gates rows 0..59 | g-gates rows 64..123]
      bank1 = [f-gates | o-gates]
    from block-diagonal replicated-weight float32r matmuls (1 cyc/row).
  - walrus rules honored: DVE same-base for two SBUF inputs (outputs and
    ACT outputs may shift partitions; PSUM input exempts the rule).
  - h-state: A [64, 416] = h0 ; B [128, 416] = [h1 | h2] at rows 0/64.
    Row 60 of A/B and row 124 of B hold constant 1.0: biases are folded
    into the recurrent matmul lhsT (bias row 60), fc bias as fc_b/SEQ.
  - FC accumulated inline over t into 2 pinned PSUM tiles [100, 416]
  - log_softmax on device (no max subtraction: logits are O(1))
"""

import sys

import numpy as np

if "/opt/trn_rl_repo" not in sys.path:
    sys.path.insert(0, "/opt/trn_rl_repo")

B_TOTAL = 32768
INPUT = 20
HID = 6
SEQ = 34
CLS = 20
NCORES = 8
BC = B_TOTAL // NCORES  # 4096
NB = 10                 # batch chunks per core
BF = 410                # batch cols per chunk
BCP = NB * BF           # 4160 padded rows per core

_CACHE = {}


# ---------------------------------------------------------------- host prep

def _build_wblob(w_ih, w_hh, b_ih, b_hh, fc_w, fc_b):
    """Pack all lhsT weight tiles into one [128, WC] fp32 blob.

    Gate order in torch weights: rows 0..5=i, 6..11=f, 12..17=g, 18..23=o.
    M-layout of IG tiles: col 6c+h = i-gate, col 64+6c+h = g-gate.
    FO tiles: f / o.  Bias row: lhsT row 60 (paired with const-1.0 row 60
    of the h rhs tiles).
    """
    cols = {}
    blocks = []
    cursor = 0

    def alloc(name, n):
        nonlocal cursor
        cols[name] = cursor
        arr = np.zeros((128, n), dtype=np.float32)
        blocks.append(arr)
        cursor += n
        return arr

    def fill_gate_cols(dst, row_of, src_w, ga, gb, nin):
        for c in range(NB):
            for h in range(HID):
                for k in range(nin):
                    r = row_of(c, k)
                    dst[r, 6 * c + h] = src_w[ga * HID + h, k]
                    dst[r, 64 + 6 * c + h] = src_w[gb * HID + h, k]

    def fill_bias_row(dst, row, bsum, ga, gb):
        for c in range(NB):
            for h in range(HID):
                dst[row, 6 * c + h] = bsum[ga * HID + h]
                dst[row, 64 + 6 * c + h] = bsum[gb * HID + h]

    bsum = [b_ih[l] + b_hh[l] for l in range(3)]

    # layer 0: x feats split 0..9 / 10..19 (chunk-major rows 10c+f), h0 tile
    for half in range(2):
        for nm, ga, gb in (("x%dIG" % half, 0, 2), ("x%dFO" % half, 1, 3)):
            a = alloc(nm, 128)
            fill_gate_cols(a, lambda c, k: 10 * c + k,
                           w_ih[0][:, half * 10:half * 10 + 10], ga, gb, 10)
    for nm, ga, gb in (("hIG0", 0, 2), ("hFO0", 1, 3)):
        a = alloc(nm, 128)
        fill_gate_cols(a, lambda c, k: 6 * c + k, w_hh[0], ga, gb, HID)
        fill_bias_row(a, 60, bsum[0], ga, gb)
    # layer 1: input part (reads A = h0, bias row) and recurrent (reads B[0:64])
    for nm, src, ga, gb, brow in (("aIG1", w_ih[1], 0, 2, True),
                                  ("aFO1", w_ih[1], 1, 3, True),
                                  ("bIG1", w_hh[1], 0, 2, False),
                                  ("bFO1", w_hh[1], 1, 3, False)):
        a = alloc(nm, 128)
        fill_gate_cols(a, lambda c, k: 6 * c + k, src, ga, gb, HID)
        if brow:
            fill_bias_row(a, 60, bsum[1], ga, gb)
    # layer 2 fused: rows 0..63 = h1 block (w_ih2, bias row 60),
    #                rows 64..127 = h2 block (w_hh2)
    for nm, ga, gb in (("W2IG", 0, 2), ("W2FO", 1, 3)):
        a = alloc(nm, 128)
        for c in range(NB):
            for h in range(HID):
                for k in range(HID):
                    a[6 * c + k, 6 * c + h] = w_ih[2][ga * HID + h, k]
                    a[6 * c + k, 64 + 6 * c + h] = w_ih[2][gb * HID + h, k]
                    a[64 + 6 * c + k, 6 * c + h] = w_hh[2][ga * HID + h, k]
                    a[64 + 6 * c + k, 64 + 6 * c + h] = w_hh[2][gb * HID + h, k]
        fill_bias_row(a, 60, bsum[2], ga, gb)
    # FC: rhs is B[64:128] (base 64) -> lhsT tiles live at rows 64..127.
    # Row 124 pairs with B's const-1.0 row: fc bias / SEQ added every t.
    for t in range(SEQ):
        a = alloc("fcA%d" % t, 100)
        b = alloc("fcB%d" % t, 100)
        for c in range(NB):
            for cl in range(10):
                for h in range(HID):
                    a[64 + 6 * c + h, 10 * c + cl] = fc_w[cl, t * HID + h]
                    b[64 + 6 * c + h, 10 * c + cl] = fc_w[10 + cl, t * HID + h]
                a[124, 10 * c + cl] = fc_b[cl] / SEQ
                b[124, 10 * c + cl] = fc_b[10 + cl] / SEQ
    # block-diag ones for per-chunk logsumexp reduce/broadcast
    a = alloc("onesK", 10)      # lhsT [100, 10]: col c = 1 at rows 10c..10c+9
    b = alloc("onesM", 100)     # lhsT [10, 100]: row c = 1 at cols 10c..10c+9
    for c in range(NB):
        a[10 * c:10 * c + 10, c] = 1.0
        b[c, 10 * c:10 * c + 10] = 1.0
    # all-ones row source for the const-1.0 rows of A/B
    a = alloc("ones416", BF)
    a[:] = 1.0

    blob = np.concatenate(blocks, axis=1)
    return np.ascontiguousarray(blob), cols


def _prep_x(x_core):
    """(4096, 20, 34) -> [34, 2, 100, 416] fp32, chunk c col j <-> row c*416+j."""
    xp = np.zeros((BCP, INPUT, SEQ), dtype=np.float32)
    xp[:BC] = x_core
    xr = xp.reshape(NB, BF, INPUT, SEQ).transpose(3, 2, 0, 1)  # (34, 20, 10, 416)
    xr = xr.reshape(SEQ, 2, 10, NB, BF).transpose(0, 1, 3, 2, 4)
    return np.ascontiguousarray(xr.reshape(SEQ, 2, 100, BF))


def _unpack_out(res):
    """[2, 100, 416] -> (4096, 20)."""
    r = res.reshape(2, NB, 10, BF)          # (half, chunk, cls, col)
    r = r.transpose(1, 3, 0, 2).reshape(BCP, CLS)
    return r[:BC]


# ---------------------------------------------------------------- program

def _make_nc(wc_total, col, loop_n=1):
    import concourse.tile as tile
    from concourse import bacc, mybir

    F = mybir.dt.float32
    FR = mybir.dt.float32r
    AF = mybir.ActivationFunctionType
    Alu = mybir.AluOpType

    nc = bacc.Bacc("TRN2", target_bir_lowering=False, debug=False)
    xd = nc.declare_dram_parameter("xin", [SEQ, 2, 100, BF], FR, isOutput=False)
    wd = nc.declare_dram_parameter("win", [128, wc_total], FR, isOutput=False)
    od = nc.declare_dram_parameter("oout", [2, 100, BF], F, isOutput=True)

    with tile.TileContext(nc) as tc:
        with (
            tc.tile_pool(name="w", bufs=1) as wp,
            tc.tile_pool(name="x", bufs=4) as xp,
            tc.tile_pool(name="s", bufs=3) as sp,
            tc.tile_pool(name="st", bufs=1) as st,
            tc.tile_pool(name="g", bufs=3, space="PSUM") as gp,
            tc.tile_pool(name="fc", bufs=1, space="PSUM") as fp,
        ):
            wsb = wp.tile([128, wc_total], FR)
            nc.sync.dma_start(out=wsb[:], in_=wd[:])

            def wap(name, r0, r1, c0, c1):
                c = col[name]
                return wsb[r0:r1, c + c0:c + c1]

            import contextlib
            loop_cm = (tc.For_i(0, loop_n, 1,
                                hint_engines=(mybir.EngineType.PE,
                                              mybir.EngineType.Activation,
                                              mybir.EngineType.DVE,
                                              mybir.EngineType.SP))
                       if loop_n > 1 else contextlib.nullcontext())
            with loop_cm:
                A = st.tile([64, BF], FR, tag="A")
                Bt = st.tile([128, BF], FR, tag="B")
                # X2[l]: bank0 = tanh(g) scratch, bank1 = c state
                X2 = [st.tile([64, 2, BF], F, tag="X2%d" % l, name="X2%d" % l)
                      for l in range(3)]
                nc.vector.memset(A[:].bitcast(F), 0.0)
                nc.vector.memset(Bt[:].bitcast(F), 0.0)
                for l in range(3):
                    nc.vector.memset(X2[l][:], 0.0)
                # const-1.0 rows (bias rows) via tiny SBUF->SBUF DMAs
                nc.sync.dma_start(out=A[60:61, :], in_=wap("ones416", 60, 61, 0, BF))
                nc.sync.dma_start(out=Bt[60:61, :], in_=wap("ones416", 60, 61, 0, BF))
                nc.sync.dma_start(out=Bt[124:125, :], in_=wap("ones416", 124, 125, 0, BF))
                pa = fp.tile([100, BF], F, tag="pa")
                pb = fp.tile([100, BF], F, tag="pb")

                hdst = {0: A[0:60], 1: Bt[0:60], 2: Bt[64:124]}
                # wavefront: stage s runs layer l at t = s - l (independent
                # chains); all matmuls first (they read last stage's h), then
                # the elementwise chains, then FC on the just-written h2.
                for s_ in range(SEQ + 2):
                    if s_ < SEQ:
                        xa = xp.tile([100, BF], FR, tag="xa")
                        xb = xp.tile([100, BF], FR, tag="xb")
                        nc.sync.dma_start(out=xa[:], in_=xd[s_, 0])
                        nc.sync.dma_start(out=xb[:], in_=xd[s_, 1])
                    rhs_sets = {
                        0: [(xa[:], "x0IG", "x0FO", 100),
                            (xb[:], "x1IG", "x1FO", 100),
                            (A[:], "hIG0", "hFO0", 64)],
                        1: [(A[:], "aIG1", "aFO1", 64),
                            (Bt[0:64], "bIG1", "bFO1", 64)],
                        2: [(Bt[:], "W2IG", "W2FO", 128)],
                    }
                    live = [l for l in range(3) if 0 <= s_ - l < SEQ]
                    g2s = {}
                    for l in live:
                        g2 = gp.tile([128, 2, 512], F, tag="g2",
                                     name="g2_%d_%d" % (s_, l))
                        g2s[l] = g2
                        items = rhs_sets[l]
                        n = len(items)
                        for gi in range(2):
                            for i, (rhs, wig, wfo, K) in enumerate(items):
                                nc.tensor.matmul(g2[:, gi, 0:BF],
                                                 wap(wig if gi == 0 else wfo,
                                                     0, K, 0, 128),
                                                 rhs,
                                                 start=(i == 0),
                                                 stop=(i == n - 1))
                    for l in live:
                        g2 = g2s[l]
                        sif = sp.tile([64, 2, BF], F, tag="sif")
                        so = sp.tile([64, BF], F, tag="so")
                        z = sp.tile([64, 2, BF], F, tag="z")
                        tcl = sp.tile([64, BF], F, tag="tcl")
                        # sigmoid(i | f) in one shot (banks 0,1 of rows 0..63)
                        nc.scalar.activation(out=sif[:], in_=g2[0:64, :, 0:BF],
                                             func=AF.Sigmoid)
                        # tanh(g): rows 64..127 bank0 -> shifted to X2 bank0
                        nc.scalar.activation(out=X2[l][:, 0, :],
                                             in_=g2[64:128, 0, 0:BF],
                                             func=AF.Tanh)
                        # sigmoid(o): rows 64..127 bank1 -> shifted to 0
                        nc.scalar.activation(out=so[:], in_=g2[64:128, 1, 0:BF],
                                             func=AF.Sigmoid)
                        # z = [i*tanh_g | f*c] in one 2-bank op
                        nc.vector.tensor_mul(out=z[:], in0=sif[:], in1=X2[l][:])
                        nc.vector.tensor_add(out=X2[l][:, 1, :],
                                             in0=z[:, 0, :], in1=z[:, 1, :])
                        nc.scalar.activation(out=tcl[:], in_=X2[l][:, 1, :],
                                             func=AF.Tanh)
                        nc.vector.tensor_mul(out=hdst[l], in0=so[0:60],
                                             in1=tcl[0:60])
                    t2_ = s_ - 2
                    if 0 <= t2_ < SEQ:
                        nc.tensor.matmul(pa[:], wap("fcA%d" % t2_, 64, 128, 0, 100),
                                         Bt[64:128],
                                         start=(t2_ == 0), stop=(t2_ == SEQ - 1))
                        nc.tensor.matmul(pb[:], wap("fcB%d" % t2_, 64, 128, 0, 100),
                                         Bt[64:128],
                                         start=(t2_ == 0), stop=(t2_ == SEQ - 1))

                # ---- log_softmax tail (logits are O(1); skip max subtraction)
                ea = sp.tile([100, BF], FR, tag="sif")
                eb = sp.tile([100, BF], FR, tag="tg")
                la = sp.tile([100, BF], F, tag="la")
                lb = sp.tile([100, BF], F, tag="lb")
                nc.scalar.activation(out=la[:], in_=pa[:], func=AF.Identity)
                nc.scalar.activation(out=lb[:], in_=pb[:], func=AF.Identity)
                nc.scalar.activation(out=ea[:], in_=pa[:], func=AF.Exp)
                nc.scalar.activation(out=eb[:], in_=pb[:], func=AF.Exp)
                s = gp.tile([10, BF], F, tag="g2")
                nc.tensor.matmul(s[:], wap("onesK", 0, 100, 0, 10), ea[:],
                                 start=True, stop=False)
                nc.tensor.matmul(s[:], wap("onesK", 0, 100, 0, 10), eb[:],
                                 start=False, stop=True)
                lnz = sp.tile([10, BF], FR, tag="lnz")
                nc.scalar.activation(out=lnz[:], in_=s[:], func=AF.Ln)
                bc = gp.tile([100, BF], F, tag="g2")
                nc.tensor.matmul(bc[:], wap("onesM", 0, 10, 0, 100), lnz[:],
                                 start=True, stop=True)
                oa = sp.tile([100, BF], F, tag="la")
                ob = sp.tile([100, BF], F, tag="lb")
                nc.vector.scalar_tensor_tensor(out=oa[:], in0=bc[:], scalar=-1.0,
                                               in1=la[:], op0=Alu.mult, op1=Alu.add)
                nc.vector.scalar_tensor_tensor(out=ob[:], in0=bc[:], scalar=-1.0,
                                               in1=lb[:], op0=Alu.mult, op1=Alu.add)
                nc.sync.dma_start(out=od[0], in_=oa[:])
                nc.sync.dma_start(out=od[1], in_=ob[:])
    nc.compile()
    return nc


def _get_program(inputs, loop_n=1):
    w_ih = [inputs["w_ih%d" % l] for l in range(3)]
    w_hh = [inputs["w_hh%d" % l] for l in range(3)]
    b_ih = [inputs["b_ih%d" % l] for l in range(3)]
    b_hh = [inputs["b_hh%d" % l] for l in range(3)]
    blob, col = _build_wblob(w_ih, w_hh, b_ih, b_hh,
                             inputs["fc_w"], inputs["fc_b"])
    key = "nc%d" % loop_n
    if key not in _CACHE:
        _CACHE[key] = _make_nc(blob.shape[1], col, loop_n)
    return _CACHE[key], blob


def kernel(**inputs):
    from concourse.bass_utils import run_bass_kernel_spmd

    nc, blob = _get_program(inputs)
    x = np.asarray(inputs["x"], dtype=np.float32)
    in_maps = []
    for c in range(NCORES):
        xc = x[c * BC:(c + 1) * BC, 0]  # (4096, 20, 34)
        in_maps.append({"xin": _prep_x(xc), "win": blob})
    res = run_bass_kernel_spmd(nc, in_maps, list(range(NCORES)),
                               trace=_CACHE.get("trace", False))
    _CACHE["last_res"] = res
    out = np.empty((B_TOTAL, CLS), dtype=np.float32)
    for c in range(NCORES):
        out[c * BC:(c + 1) * BC] = _unpack_out(res.results[c]["oout"])
    return out



# revision 2
# speedup vs baseline: 1.2309x; 1.2309x over previous
"""Trainium2 Bass kernel v2 for nn_BasicRNN_42271068127787.

3-layer LSTM (input=20, hidden=6, seq=34) + FC(204->20) + log_softmax over
batch 32768, data-parallel over 8 NeuronCores (4096 rows/core).

v2 layout (per core), all bf16 compute / fp32 psum:
  - batch 4096 -> NB=20 chunks x BF=208 cols (4160 padded rows)
  - partition map P(c,h) = 6*(c%10) + h + 64*(c//10):
    chunks 0-9 at rows 0-59, chunks 10-19 at rows 64-123 (legal base
    partitions 0/64 for all sliced engine ops), bias const row 124.
  - gates: single-gate matmuls M=128 (pad cols zero), N=208, bf16
    (1 cyc/row at any N). PSUM: GIF [128,3,2,256] = 3 banks (layer
    l bank, i at 0 / f at 1024B), GGO same for g|o, FC [100,2,512].
  - elementwise: 4 ACT + 3 DVE ops per stage, 3 layers consolidated
    via strided multi-bank APs at 120-of-128-partition density.
  - L0 x-projection matmuls run one stage ahead (JIT) to fill PE.
  - FC in NB10-416 form (2 matmuls/stage) on h2r staged by Pool copies.
  - log_softmax tail on device as in v1.
"""

import sys

import numpy as np
import ml_dtypes

if "/opt/trn_rl_repo" not in sys.path:
    sys.path.insert(0, "/opt/trn_rl_repo")

BF16NP = ml_dtypes.bfloat16

B_TOTAL = 32768
INPUT = 20
HID = 6
SEQ = 34
CLS = 20
NCORES = 8
BC = B_TOTAL // NCORES   # 4096
NB = 20                  # chunks per core
BFW = 208                # batch cols per chunk
BCP = NB * BFW           # 4160

_CACHE = {}


def _prow(c, h):
    """partition row of (chunk c, hidden h)"""
    return 6 * (c % 10) + h + 64 * (c // 10)


# ---------------------------------------------------------------- host prep

def _build_wblob(w_ih, w_hh, b_ih, b_hh, fc_w, fc_b):
    """Pack all lhsT tiles into one [128, WC] bf16 blob. Returns (blob, cols).

    Column map (all M-cols use _prow pad-zero convention):
      xk{kt}{g}   [K_kt,128] : L0 x-proj, K-row r -> xrow kt*128+r = c*20+f
      h{l}{src}{g}[128,128]  : h-part lhsT; src 0=input-h, 1=recurrent.
                               bias folded at row 124 of src-0 (src-1 for l=0)
      fcA{t}/fcB{t} [61,100] : FC lhsT per t, bias row 60 (fc_b/SEQ)
      onesK [100,10], onesM [10,100], ones [128, 624] (const 1.0 rows)
    """
    cols = {}
    blocks = []
    cursor = 0

    def alloc(name, n):
        nonlocal cursor
        cols[name] = cursor
        arr = np.zeros((128, n), dtype=np.float32)
        blocks.append(arr)
        cursor += n
        return arr

    KT_SIZES = [128, 128, 128, 16]
    for kt in range(4):
        for g in range(4):
            a = alloc("xk%d%d" % (kt, g), 128)
            for r in range(KT_SIZES[kt]):
                xrow = kt * 128 + r
                c, f = divmod(xrow, INPUT)
                for h in range(HID):
                    a[r, _prow(c, h)] = w_ih[0][g * HID + h, f]

    bsum = [b_ih[l] + b_hh[l] for l in range(3)]
    # (layer, src) -> (weight, bias?)
    hsrc = {
        (0, 1): (w_hh[0], True),
        (1, 0): (w_ih[1], True), (1, 1): (w_hh[1], False),
        (2, 0): (w_ih[2], True), (2, 1): (w_hh[2], False),
    }
    for (l, src), (w, has_b) in hsrc.items():
        for g in range(4):
            a = alloc("h%d%d%d" % (l, src, g), 128)
            for c in range(NB):
                for hh in range(HID):      # input row (h')
                    for h in range(HID):   # output col (h)
                        a[_prow(c, hh), _prow(c, h)] = w[g * HID + h, hh]
            if has_b:
                for c in range(NB):
                    for h in range(HID):
                        a[124, _prow(c, h)] = bsum[l][g * HID + h]

    for t in range(SEQ):
        a = alloc("fcA%d" % t, 100)
        b = alloc("fcB%d" % t, 100)
        for cp in range(10):
            for cl in range(10):
                for h in range(HID):
                    a[6 * cp + h, 10 * cp + cl] = fc_w[cl, t * HID + h]
                    b[6 * cp + h, 10 * cp + cl] = fc_w[10 + cl, t * HID + h]
                a[60, 10 * cp + cl] = fc_b[cl] / SEQ
                b[60, 10 * cp + cl] = fc_b[10 + cl] / SEQ

    a = alloc("onesK", 10)
    b = alloc("onesM", 100)
    for cp in range(10):
        a[10 * cp:10 * cp + 10, cp] = 1.0
        b[cp, 10 * cp:10 * cp + 10] = 1.0
    a = alloc("ones", 624)
    a[:] = 1.0

    blob = np.concatenate(blocks, axis=1)
    return np.ascontiguousarray(blob.astype(BF16NP)), cols


def _prep_x(x_core):
    """(4096, 20, 34) -> [34, 128, 4, 208] bf16; xrow = c*20+f = kt*128+p."""
    xp = np.zeros((BCP, INPUT, SEQ), dtype=np.float32)
    xp[:BC] = x_core
    # (chunk, col, feat, t) -> xrow (c*20+f), col
    xr = xp.reshape(NB, BFW, INPUT, SEQ).transpose(3, 0, 2, 1)  # (t, c, f, col)
    xr = xr.reshape(SEQ, NB * INPUT, BFW)                       # (t, xrow, col)
    out = np.zeros((SEQ, 4, 128, BFW), dtype=np.float32)
    out.reshape(SEQ, 512, BFW)[:, :400] = xr
    return np.ascontiguousarray(out.transpose(0, 2, 1, 3).astype(BF16NP))


def _unpack_out(res):
    """[2, 100, 416] fp32 -> (4096, 20): row 10*cp+cl, col j ->
    batch (cp + 10*(j>=208))*208 + j%208, class half*10+cl."""
    r = np.asarray(res, dtype=np.float32).reshape(2, 10, 10, 2, BFW)
    # dims: half, cp, cl, jhi, b  ->  batch chunk = cp + 10*jhi
    r = r.transpose(3, 1, 4, 0, 2)          # (jhi, cp, b, half, cl)
    r = r.reshape(NB * BFW, CLS, order="C")  # ((jhi,cp),b) -> chunk-major rows
    # careful: (jhi, cp) ordering gives chunk = jhi*10+cp -- fix below
    out = np.empty((BCP, CLS), dtype=np.float32)
    rr = r.reshape(2, 10, BFW, CLS)
    for jhi in range(2):
        for cp in range(10):
            c = cp + 10 * jhi
            out[c * BFW:(c + 1) * BFW] = rr[jhi, cp]
    return out[:BC]


# ---------------------------------------------------------------- program

def _make_nc(wc_total, col):
    import concourse.tile as tile
    from concourse import bacc, mybir

    F = mybir.dt.float32
    BT = mybir.dt.bfloat16
    AF = mybir.ActivationFunctionType
    Alu = mybir.AluOpType

    KT_SIZES = [128, 128, 128, 16]

    nc = bacc.Bacc("TRN2", target_bir_lowering=False, debug=False)
    xd = nc.declare_dram_parameter("xin", [SEQ, 128, 4, BFW], BT, isOutput=False)
    wd = nc.declare_dram_parameter("win", [128, wc_total], BT, isOutput=False)
    od = nc.declare_dram_parameter("oout", [2, 100, 416], F, isOutput=True)

    with tile.TileContext(nc) as tc:
        with (
            tc.tile_pool(name="w", bufs=1) as wp,
            tc.tile_pool(name="st", bufs=1) as st,
            tc.tile_pool(name="g", bufs=1, space="PSUM") as gp,
        ):
            ws = wp.tile([128, wc_total], BT)
            nc.sync.dma_start(out=ws[:], in_=wd[:])

            def wap(name, r0, r1, c0, c1):
                c = col[name]
                return ws[r0:r1, c + c0:c + c1]

            xs = st.tile([128, SEQ, 4, BFW], BT, tag="xs")
            for s in range(SEQ):
                nc.sync.dma_start(out=xs[:, s, :, :], in_=xd[s])

            # persistent state
            hall = st.tile([128, 3, BFW], BT, tag="hall")   # h0|h1|h2
            gc = st.tile([128, 3, 2, BFW], BT, tag="gc")    # (l,0)=tanh g,(l,1)=c
            h2r = st.tile([64, 416], BT, tag="h2r")         # h2, NB10-416 form
            nc.vector.memset(hall[:], 0.0)
            nc.vector.memset(gc[:], 0.0)
            nc.vector.memset(h2r[:], 0.0)
            # const-1.0 rows via DMA (partition 124 / 60 not memset-legal)
            nc.sync.dma_start(out=hall[124:125, :, :],
                              in_=wap("ones", 124, 125, 0, 3 * BFW))
            nc.sync.dma_start(out=h2r[60:61, :], in_=wap("ones", 60, 61, 0, 416))

            # working tiles
            sif = st.tile([128, 3, 2, BFW], BT, tag="sif")
            osg = st.tile([128, 3, BFW], BT, tag="osg")
            zt = st.tile([128, 3, 2, BFW], BT, tag="zt")
            tct = st.tile([128, 3, BFW], BT, tag="tct")

            # psum: 3 + 3 + 2 banks
            gif = gp.tile([128, 3, 2, 256], F, tag="gif")
            ggo = gp.tile([128, 3, 2, 256], F, tag="ggo")
            fcp = gp.tile([100, 2, 512], F, tag="fcp")

            def emit_xproj(t):
                # L0 x-projection for stage t. One accumulation group per
                # PSUM BANK per stage: the first matmul of the bank epoch
                # carries start=True (whole-bank lazy zero); each region's
                # first write auto-zeroes; the last h-part carries stop.
                for g, dst in ((0, gif), (1, gif), (2, ggo), (3, ggo)):
                    bank = g % 2
                    for kt in range(4):
                        k = KT_SIZES[kt]
                        nc.tensor.matmul(
                            dst[:, 0, bank, 0:BFW],
                            wap("xk%d%d" % (kt, g), 0, k, 0, 128),
                            xs[0:k, t, kt, :],
                            start=(g % 2 == 0 and kt == 0), stop=False)

            def _hmm(l, src, g, start, stop):
                dst = gif if g < 2 else ggo
                hb = l if src else l - 1
                name = "h01%d" % g if l == 0 else "h%d%d%d" % (l, src, g)
                nc.tensor.matmul(dst[:, l, g % 2, 0:BFW],
                                 wap(name, 0, 128, 0, 128),
                                 hall[:, hb, :], start=start, stop=stop)

            def emit_hparts(t):
                # h-dependent gate matmuls for stage t (need h[t-1]).
                # if-banks first so sigmoid(if) unblocks earliest. One
                # group per bank per stage: L0's close bank-0 epochs
                # (opened by xproj); L1/L2 open+close their own banks.
                live_t = [l for l in range(3) if 0 <= t - l < SEQ]
                for gates in ((0, 1), (2, 3)):
                    for l in live_t:
                        for g in gates:
                            if l == 0:
                                _hmm(0, 1, g, False, g % 2 == 1)
                            else:
                                _hmm(l, 0, g, g % 2 == 0, False)
                                _hmm(l, 1, g, False, g % 2 == 1)

            emit_xproj(0)
            emit_hparts(0)

            for s in range(SEQ + 2):
                live = [l for l in range(3) if 0 <= s - l < SEQ]
                l0, l1 = live[0], live[-1] + 1
                t2 = s - 2

                # --- activations on gates
                nc.scalar.activation(out=sif[:, l0:l1, :, :],
                                     in_=gif[:, l0:l1, :, 0:BFW],
                                     func=AF.Sigmoid)
                nc.scalar.activation(out=gc[:, l0:l1, 0, :],
                                     in_=ggo[:, l0:l1, 0, 0:BFW],
                                     func=AF.Tanh)
                nc.scalar.activation(out=osg[:, l0:l1, :],
                                     in_=ggo[:, l0:l1, 1, 0:BFW],
                                     func=AF.Sigmoid)

                # --- next stage's x-projection: unblocks as soon as the
                # gate reads above retire; keeps PE busy through the chain
                if s + 1 < SEQ:
                    emit_xproj(s + 1)

                # --- cell update
                nc.vector.tensor_mul(out=zt[:, l0:l1, :, :],
                                     in0=sif[:, l0:l1, :, :],
                                     in1=gc[:, l0:l1, :, :])
                nc.vector.tensor_add(out=gc[:, l0:l1, 1, :],
                                     in0=zt[:, l0:l1, 0, :],
                                     in1=zt[:, l0:l1, 1, :])
                nc.scalar.activation(out=tct[:, l0:l1, :],
                                     in_=gc[:, l0:l1, 1, :],
                                     func=AF.Tanh)
                nc.vector.tensor_mul(out=hall[0:124, l0:l1, :],
                                     in0=osg[0:124, l0:l1, :],
                                     in1=tct[0:124, l0:l1, :])

                # --- Pool stages h2 into NB10-416 form for FC (t2 = s-2)
                if 0 <= t2 < SEQ:
                    nc.gpsimd.tensor_copy(out=h2r[0:60, 0:BFW],
                                          in_=hall[0:60, 2, :])
                    nc.gpsimd.tensor_copy(out=h2r[0:60, BFW:416],
                                          in_=hall[64:124, 2, :])

                # --- h-dependent gate matmuls for the next stage
                if s + 1 < SEQ + 2:
                    emit_hparts(s + 1)

                # --- FC matmuls last so they never head-of-line block the
                # next stage's gate matmuls
                if 0 <= t2 < SEQ:
                    nc.tensor.matmul(fcp[:, 0, 0:416],
                                     wap("fcA%d" % t2, 0, 61, 0, 100),
                                     h2r[0:61, :],
                                     start=(t2 == 0), stop=(t2 == SEQ - 1))
                    nc.tensor.matmul(fcp[:, 1, 0:416],
                                     wap("fcB%d" % t2, 0, 61, 0, 100),
                                     h2r[0:61, :],
                                     start=(t2 == 0), stop=(t2 == SEQ - 1))

            # ---- log_softmax tail (logits O(1); skip max subtraction)
            la = st.tile([100, 416], F, tag="la")
            lb = st.tile([100, 416], F, tag="lb")
            ea = st.tile([100, 416], BT, tag="ea")
            eb = st.tile([100, 416], BT, tag="eb")
            nc.scalar.activation(out=la[:], in_=fcp[:, 0, 0:416], func=AF.Identity)
            nc.scalar.activation(out=lb[:], in_=fcp[:, 1, 0:416], func=AF.Identity)
            nc.scalar.activation(out=ea[:], in_=fcp[:, 0, 0:416], func=AF.Exp)
            nc.scalar.activation(out=eb[:], in_=fcp[:, 1, 0:416], func=AF.Exp)
            sm = gp.tile([10, 416], F, tag="gif")
            nc.tensor.matmul(sm[:], wap("onesK", 0, 100, 0, 10), ea[:],
                             start=True, stop=False)
            nc.tensor.matmul(sm[:], wap("onesK", 0, 100, 0, 10), eb[:],
                             start=False, stop=True)
            lnz = st.tile([10, 416], BT, tag="lnz")
            nc.scalar.activation(out=lnz[:], in_=sm[:], func=AF.Ln)
            bc = gp.tile([100, 416], F, tag="ggo")
            nc.tensor.matmul(bc[:], wap("onesM", 0, 10, 0, 100), lnz[:],
                             start=True, stop=True)
            oa = st.tile([100, 416], F, tag="oa")
            ob = st.tile([100, 416], F, tag="ob")
            nc.vector.scalar_tensor_tensor(out=oa[:], in0=bc[:], scalar=-1.0,
                                           in1=la[:], op0=Alu.mult, op1=Alu.add)
            nc.vector.scalar_tensor_tensor(out=ob[:], in0=bc[:], scalar=-1.0,
                                           in1=lb[:], op0=Alu.mult, op1=Alu.add)
            nc.sync.dma_start(out=od[0], in_=oa[:])
            nc.sync.dma_start(out=od[1], in_=ob[:])
    nc.compile()
    return nc


def _get_program(inputs):
    w_ih = [np.asarray(inputs["w_ih%d" % l], np.float32) for l in range(3)]
    w_hh = [np.asarray(inputs["w_hh%d" % l], np.float32) for l in range(3)]
    b_ih = [np.asarray(inputs["b_ih%d" % l], np.float32) for l in range(3)]
    b_hh = [np.asarray(inputs["b_hh%d" % l], np.float32) for l in range(3)]
    blob, col = _build_wblob(w_ih, w_hh, b_ih, b_hh,
                             np.asarray(inputs["fc_w"], np.float32),
                             np.asarray(inputs["fc_b"], np.float32))
    if "nc" not in _CACHE:
        _CACHE["nc"] = _make_nc(blob.shape[1], col)
    return _CACHE["nc"], blob


def kernel(**inputs):
    from concourse.bass_utils import run_bass_kernel_spmd

    nc, blob = _get_program(inputs)
    x = np.asarray(inputs["x"], dtype=np.float32)
    in_maps = []
    for c in range(NCORES):
        xc = x[c * BC:(c + 1) * BC, 0]
        in_maps.append({"xin": _prep_x(xc), "win": blob})
    res = run_bass_kernel_spmd(nc, in_maps, list(range(NCORES)),
                               trace=_CACHE.get("trace", False))
    _CACHE["last_res"] = res
    out = np.empty((B_TOTAL, CLS), dtype=np.float32)
    for c in range(NCORES):
        out[c * BC:(c + 1) * BC] = _unpack_out(res.results[c]["oout"])
    return out


# ---------------------------------------------------------------- local check

if __name__ == "__main__":
    import jax

    sys.path.insert(0, "/root/problem")
    import reference as R
    from concourse.bass_interp import CoreSim

    inputs = {k: np.asarray(v) for k, v in R.setup_inputs().items()}
    nc, blob = _get_program(inputs)
    print("COMPILE OK")

    with jax.default_device(jax.devices("cpu")[0]):
        expected = np.asarray(jax.jit(R.reference, backend="cpu")(**inputs))

    xc = inputs["x"][:BC, 0]
    sim = CoreSim(nc)
    sim.tensor("xin")[:] = _prep_x(xc)
    sim.tensor("win")[:] = blob
    sim.simulate()
    got = _unpack_out(sim.tensor("oout"))
    err = np.abs(got - expected[:BC])
    print("abs max err:", err.max())
    print("rel err:", err.max() / np.abs(expected).max())
